# revision 1
# baseline (speedup 1.0000x reference)
"""Multi-head attention (B=4, S=2048, D=1024, H=16) on 8 TRN2 NeuronCores.

Sharding (data + head parallel): core c handles batch b = c//2 and head
group g = c%2 (8 of the 16 heads, feature columns 512g:512(g+1)).
Each core computes its heads' full attention locally and a partial
output projection; the host sums the two partials per batch and adds
b_o plus the b_v @ W_o term (softmax rows sum to 1, so the V bias is an
exact constant output offset and never touches the device).

v3 schedule (ScalarE exp is the steady-state bottleneck at ~1.03us per
[128,1024] tile, 256 tiles = 264us; everything else must hide under it):
  - Head: per seq-chunk jc, DMA xk-jc then all 4 K-proj pbs, then the
    first group's scores j-blocks for that chunk, so exp starts as
    soon as wk+xk-jc0+wq+xq-jc0 have landed (~15us) instead of after
    the full K projection (~70us).
  - Q projection just-in-time per (pair, ic) group from fully staged
    xq; QT and AT are rolling [128,512] buffers.
  - scores^T per j-block: two K=64 row-packed matmuls (2 heads),
    exp on ScalarE from PSUM (scale=1/8 folded; no max subtraction:
    scores ~ N(0,1) so exp is safely bounded).
  - PV per head: V augmented with a ones column (M=65) so PSUM row 64
    accumulates the softmax denominator; evacuate, reciprocal +
    gpsimd partition_broadcast, multiply into the rolling AT tile.
  - out = Wo^T @ AT per ic, fp32 partial to HBM, emitted one group
    after each ic's last pair so only ic3's projection trails the
    last exp.
"""

import os

import numpy as np

import concourse.bass as bass  # noqa: F401
import concourse.mybir as mybir
import concourse.tile as tile
from concourse import bacc
from concourse.bass_utils import run_bass_kernel_spmd

f32 = mybir.dt.float32
bf16 = mybir.dt.bfloat16
fp8 = mybir.dt.float8e4
Exp = mybir.ActivationFunctionType.Exp
MULT = mybir.AluOpType.mult
ADD = mybir.AluOpType.add
i32 = mybir.dt.int32

# Schraudolph fast-exp: bitcast(int32(A*s + BC)) ~ exp(s/8), max rel ~3%
SCHRAUD_A = float((2 ** 23) / np.log(2.0) * 0.125)
SCHRAUD_B = float(127.0 * (2 ** 23) - 366392.5)

B, S, D = 4, 2048, 1024
H_LOC = 8
DK = 64
DG = 512
KB = D // 128
PB = DG // 128
JB = S // 128
IC = S // 512
N = 512
QK_DT = bf16


def _build():
    nc = bacc.Bacc("TRN2")

    xq = nc.dram_tensor("xq", (D, S), QK_DT, kind="ExternalInput")
    xk = nc.dram_tensor("xk", (D, S), QK_DT, kind="ExternalInput")
    xv = nc.dram_tensor("xv", (D, S), bf16, kind="ExternalInput")
    wq = nc.dram_tensor("wq", (D, DG), QK_DT, kind="ExternalInput")
    wk = nc.dram_tensor("wk", (D, DG), QK_DT, kind="ExternalInput")
    wv = nc.dram_tensor("wv", (D, DG), bf16, kind="ExternalInput")
    wo = nc.dram_tensor("wo", (DG, D), bf16, kind="ExternalInput")
    bq = nc.dram_tensor("bq", (DG,), f32, kind="ExternalInput")
    bk = nc.dram_tensor("bk", (DG,), f32, kind="ExternalInput")
    o_t = nc.dram_tensor("o_t", (D, S), f32, kind="ExternalOutput")

    with tile.TileContext(nc) as tc:
        with (
            tc.tile_pool(name="persist", bufs=1) as persist,
            tc.tile_pool(name="wp", bufs=3) as wp,
            tc.tile_pool(name="xp", bufs=10) as xp,
            tc.tile_pool(name="xvp", bufs=12) as xvp,
            tc.tile_pool(name="qtp", bufs=4) as qtp,
            tc.tile_pool(name="atp", bufs=6) as atp,
            tc.tile_pool(name="ptp", bufs=29) as ptp,
            tc.tile_pool(name="pvs", bufs=2) as pvsp,
            tc.tile_pool(name="rbp", bufs=2) as rbp,
            tc.tile_pool(name="i32p", bufs=2) as i32p,
            tc.tile_pool(name="osb", bufs=2) as osbp,
            tc.tile_pool(name="sps", bufs=3, space="PSUM") as sps,
            tc.tile_pool(name="mps", bufs=2, space="PSUM") as mps,
        ):
            # ---- persistent tensors -------------------------------------
            KT = [persist.tile([128, S], QK_DT, tag=f"kt{p}", name=f"kt{p}")
                  for p in range(PB)]
            VA = [persist.tile([128, H_LOC, DK + 1], bf16, tag=f"va{j}",
                               name=f"va{j}") for j in range(JB)]
            xq_sb = persist.tile([128, KB, S], QK_DT, tag="xqs", name="xq_sb")
            for j in range(JB):
                nc.vector.memset(VA[j][:, :, DK:DK + 1], 1.0)

            nbias = persist.tile([128, 1], f32, tag="nbias", name="nbias")
            nc.vector.memset(nbias, -2.0)

            bq_t = persist.tile([128, PB], f32, tag="bq")
            bk_t = persist.tile([128, PB], f32, tag="bk")
            nc.sync.dma_start(out=bq_t, in_=bq.rearrange("(pb p) -> p pb", p=128))
            nc.sync.dma_start(out=bk_t, in_=bk.rearrange("(pb p) -> p pb", p=128))

            # ---- DMA staging -------------------------------------------
            # wk first, per-jc xk chunks DMA'd inside the head loop right
            # before their K-proj consumers; wq + xq-jc0 early so the
            # first scores group starts ~15us in; xv inside v_proj_chunk;
            # wo allocated after all k_proj reads are emitted (pool
            # rotation reuses wk's buffer).
            wk_t = wp.tile([128, KB, N], QK_DT, tag="w", name="wk_t")
            nc.sync.dma_start(
                out=wk_t, in_=wk.rearrange("(kb p) n -> p kb n", p=128)
            )
            wq_t = wp.tile([128, KB, N], QK_DT, tag="w", name="wq_t")

            def dma_xk_chunk(jc):
                tiles = []
                for kb in range(KB):
                    t = xp.tile([128, N], QK_DT, tag="xc", name="xc_k")
                    nc.sync.dma_start(
                        out=t,
                        in_=xk[kb * 128:(kb + 1) * 128, jc * N:(jc + 1) * N],
                    )
                    tiles.append(t)
                return tiles

            def dma_xq_chunk(jc):
                for kb in range(KB):
                    nc.sync.dma_start(
                        out=xq_sb[:, kb, jc * N:(jc + 1) * N],
                        in_=xq[kb * 128:(kb + 1) * 128, jc * N:(jc + 1) * N],
                    )

            # ---- projection helpers ------------------------------------
            def k_proj(jc, pb, xc):
                ps = mps.tile([128, N], f32, tag="mm", name="ps_k")
                for kb in range(KB):
                    nc.tensor.matmul(
                        ps,
                        wk_t[:, kb, pb * 128:(pb + 1) * 128],
                        xc[kb],
                        start=(kb == 0),
                        stop=(kb == KB - 1),
                    )
                nc.vector.tensor_scalar_add(
                    KT[pb][:, jc * N:(jc + 1) * N], ps, bk_t[:, pb:pb + 1]
                )

            def q_proj(p, ic):
                ps = mps.tile([128, N], f32, tag="mm", name="ps_q")
                for kb in range(KB):
                    nc.tensor.matmul(
                        ps,
                        wq_t[:, kb, p * 128:(p + 1) * 128],
                        xq_sb[:, kb, ic * N:(ic + 1) * N],
                        start=(kb == 0),
                        stop=(kb == KB - 1),
                    )
                qt = qtp.tile([128, N], QK_DT, tag="qt", name="qt")
                nc.vector.tensor_scalar_add(qt, ps, bq_t[:, p:p + 1])
                return qt

            def dma_xv_chunk(jg):
                xc = []
                for kb in range(KB):
                    t = xvp.tile([128, N], bf16, tag="xcv", name="xcv")
                    nc.sync.dma_start(
                        out=t,
                        in_=xv[kb * 128:(kb + 1) * 128, jg * N:(jg + 1) * N],
                    )
                    xc.append(t)
                return xc

            def v_proj_chunk(jg, xc=None):
                if xc is None:
                    xc = dma_xv_chunk(jg)
                for jj in range(4):
                    j = jg * 4 + jj
                    ps = mps.tile([128, N], f32, tag="mm", name="vps")
                    for kb in range(KB):
                        nc.tensor.matmul(
                            ps,
                            xc[kb][:, jj * 128:(jj + 1) * 128],
                            wv_t[:, kb, :],
                            start=(kb == 0),
                            stop=(kb == KB - 1),
                        )
                    nc.vector.tensor_copy(
                        VA[j][:, :, 0:DK],
                        ps.rearrange("p (h e) -> p h e", e=DK),
                    )

            # ---- attention phases --------------------------------------
            def scores_block(pair, j, qt, fast=False):
                s_ps = sps.tile([128, 2 * N], f32, tag="s", name="s_ps")
                nc.tensor.matmul(
                    s_ps[:, 0:N],
                    KT[pair][0:64, j * 128:(j + 1) * 128],
                    qt[0:64, :],
                    start=True, stop=True,
                )
                nc.tensor.matmul(
                    s_ps[:, N:2 * N],
                    KT[pair][64:128, j * 128:(j + 1) * 128],
                    qt[64:128, :],
                    start=True, stop=True,
                    tile_position=(64, 0),
                )
                pt = ptp.tile([128, 2 * N], bf16, tag="pt", name="pt")
                if fast:
                    # Schraudolph on DVE relieves the ScalarE exp floor
                    it = i32p.tile([128, 2 * N], i32, tag="it", name="it")
                    nc.vector.tensor_scalar(
                        out=it, in0=s_ps, scalar1=SCHRAUD_A,
                        scalar2=SCHRAUD_B, op0=MULT, op1=ADD,
                    )
                    nc.vector.tensor_copy(pt, it.bitcast(f32))
                else:
                    nc.scalar.activation(pt, s_ps, Exp, scale=0.125)
                return pt

            FAST_J = ()

            def scores_group(pair, ic, qt, jset=range(JB), pts=None):
                # DVE fast-exp only in the steady-state (ic>=1) groups:
                # the ic0 phase is PE/DMA-bound with VectorE latency on
                # the critical path, where extra DVE work backfires.
                if pts is None:
                    pts = []
                for j in jset:
                    fast = ic >= 1 and j in FAST_J
                    pts.append(scores_block(pair, j, qt, fast=fast))
                return (pair, ic, pts)

            def pv_phase(pair, ic, pts, last=False):
                pv = [
                    mps.tile([DK + 1, N], f32, tag="mm", name="pv0"),
                    mps.tile([DK + 1, N], f32, tag="mm", name="pv1"),
                ]
                if last:
                    # j-outer so only the final j's matmuls trail the
                    # last exp instead of a whole serial h2=1 pass
                    for j in range(JB):
                        for h2 in range(2):
                            nc.tensor.matmul(
                                pv[h2],
                                VA[j][:, 2 * pair + h2, :],
                                pts[j][:, h2 * N:(h2 + 1) * N],
                                start=(j == 0),
                                stop=(j == JB - 1),
                                skip_group_check=True,
                            )
                else:
                    for h2 in range(2):
                        for j in range(JB):
                            nc.tensor.matmul(
                                pv[h2],
                                VA[j][:, 2 * pair + h2, :],
                                pts[j][:, h2 * N:(h2 + 1) * N],
                                start=(j == 0),
                                stop=(j == JB - 1),
                            )
                at = atp.tile([128, N], bf16, tag="at", name="at")
                for h2 in range(2):
                    pvs = pvsp.tile([DK + 1, N], f32, tag="pvs", name="pvs")
                    nc.vector.tensor_copy(pvs[0:DK, :], pv[h2][0:DK, :])
                    den = rbp.tile([1, N], f32, tag="den", name="den")
                    nc.vector.tensor_copy(den, pv[h2][DK:DK + 1, :])
                    rbr = rbp.tile([64, N], f32, tag="rbr", name="rbr")
                    nc.gpsimd.partition_broadcast(rbr, den)
                    rb = rbp.tile([64, N], f32, tag="rb", name="rb")
                    nc.vector.reciprocal_approx_fast(rb, rbr)
                    nc.vector.tensor_tensor(
                        out=at[h2 * 64:(h2 + 1) * 64, :],
                        in0=pvs[0:DK, :], in1=rb, op=MULT,
                    )
                return at

            def oproj_ic(ic, at_tiles, dobs=None, evac_scalar=False):
                for dob in (range(KB) if dobs is None else dobs):
                    ops = mps.tile([128, N], f32, tag="mm", name="ops")
                    for pb in range(PB):
                        nc.tensor.matmul(
                            ops,
                            wo_t[:, pb, dob * 128:(dob + 1) * 128],
                            at_tiles[pb],
                            start=(pb == 0),
                            stop=(pb == PB - 1),
                        )
                    ob = osbp.tile([128, N], f32, tag="ob", name="ob")
                    if evac_scalar:
                        nc.scalar.copy(ob, ops)
                    else:
                        nc.vector.tensor_copy(ob, ops)
                    nc.sync.dma_start(
                        out=o_t[dob * 128:(dob + 1) * 128, ic * N:(ic + 1) * N],
                        in_=ob,
                    )

            # ---- main schedule -----------------------------------------
            # Head: wk + xk-jc0 stream first; per seq-chunk jc, all 4
            # K-proj pbs then the first group's scores j-blocks for
            # that chunk. wq + xq-jc0 queue behind xk-jc0 so Q(0,0)
            # lands right after K(jc0).
            groups = [(p, ic) for ic in range(IC) for p in range(PB)]
            qt_next = {}
            st = None
            for jc in range(IC):
                xc = dma_xk_chunk(jc)
                if jc == 0:
                    nc.sync.dma_start(
                        out=wq_t, in_=wq.rearrange("(kb p) n -> p kb n", p=128)
                    )
                    for kb in range(KB):
                        nc.sync.dma_start(
                            out=xq_sb[:, kb, 0:N],
                            in_=xq[kb * 128:(kb + 1) * 128, 0:N],
                        )
                for pb in range(PB):
                    k_proj(jc, pb, xc)
                if jc == 0:
                    qt_next[(0, 0)] = q_proj(0, 0)
                    st = scores_group(0, 0, qt_next[(0, 0)], range(0, 4))
                else:
                    st = scores_group(0, 0, qt_next[(0, 0)],
                                      range(4 * jc, 4 * jc + 4), pts=st[2])

            # post-head DMA: wv + the first two xv chunks stream before
            # the remaining xq chunks so V-proj isn't starved.
            wv_t = wp.tile([128, KB, N], bf16, tag="w", name="wv_t")
            nc.sync.dma_start(
                out=wv_t, in_=wv.rearrange("(kb p) n -> p kb n", p=128)
            )
            xv01 = [dma_xv_chunk(0), dma_xv_chunk(1)]
            dma_xq_chunk(1)
            qt_next[(1, 0)] = q_proj(1, 0)

            wo_t = None
            prev = None        # unconsumed (pair, ic, pts)
            at_done = {}       # (pair, ic) -> at tile
            for gi, (pair, ic) in enumerate(groups):
                if (pair, ic) != (0, 0):
                    st = scores_group(pair, ic, qt_next.pop((pair, ic)))
                # q-proj one group ahead, right behind this group's scores
                nxt = groups[gi + 1] if gi + 1 < len(groups) else None
                if nxt is not None and nxt not in qt_next:
                    qt_next[nxt] = q_proj(*nxt)
                # ---- PE filler inside this group's exp window ----
                if ic == 0:
                    if pair == 0:
                        v_proj_chunk(0, xv01[0])
                        v_proj_chunk(1, xv01[1])
                    elif pair == 1:
                        # xv chunks 2/3 DMA first; xq-jc2/3 are not
                        # needed until the ic2/ic3 q-projections
                        v_proj_chunk(2)
                        v_proj_chunk(3)
                        dma_xq_chunk(2)
                        dma_xq_chunk(3)
                    elif pair == 2:
                        # wo reuses wk's pool buffer; allocate only after
                        # every k_proj read has been emitted.
                        wo_t = wp.tile([128, PB, D], bf16, tag="w",
                                       name="wo_t")
                        nc.sync.dma_start(
                            out=wo_t,
                            in_=wo.rearrange("(pb p) n -> p pb n", p=128),
                        )
                if prev is not None:
                    p_prev, ic_prev = prev[0], prev[1]
                    at_done[(p_prev, ic_prev)] = pv_phase(*prev)
                    if ic > 0:
                        # spread the output projection of ic-1 across this
                        # ic's groups so no single exp window overruns
                        ats = [at_done[(p, ic - 1)] for p in range(PB)]
                        if pair == 1:
                            oproj_ic(ic - 1, ats, range(0, 3))
                        elif pair == 2:
                            oproj_ic(ic - 1, ats, range(3, 6))
                        elif pair == 3:
                            oproj_ic(ic - 1, ats, range(6, 8))
                            for p in range(PB):
                                at_done.pop((p, ic - 1))
                prev = st
            at_done[(prev[0], prev[1])] = pv_phase(*prev)
            oproj_ic(IC - 1, [at_done.pop((p, IC - 1)) for p in range(PB)])

    nc.compile()
    return nc


_NC_CACHE = None


def _get_nc():
    global _NC_CACHE
    if _NC_CACHE is None:
        _NC_CACHE = _build()
    return _NC_CACHE


def kernel(q, k, v, W_q, b_q, W_k, b_k, W_v, b_v, W_o, b_o):
    import ml_dtypes

    q = np.asarray(q, dtype=np.float32)
    k = np.asarray(k, dtype=np.float32)
    v = np.asarray(v, dtype=np.float32)
    W_q = np.asarray(W_q, dtype=np.float32)
    W_k = np.asarray(W_k, dtype=np.float32)
    W_v = np.asarray(W_v, dtype=np.float32)
    W_o = np.asarray(W_o, dtype=np.float32)
    b_q = np.asarray(b_q, dtype=np.float32)
    b_k = np.asarray(b_k, dtype=np.float32)
    b_v = np.asarray(b_v, dtype=np.float32)
    b_o = np.asarray(b_o, dtype=np.float32)

    bf = ml_dtypes.bfloat16
    xq_t = [np.ascontiguousarray(q[b].T).astype(bf) for b in range(B)]
    xk_t = [np.ascontiguousarray(k[b].T).astype(bf) for b in range(B)]
    xv_t = [np.ascontiguousarray(v[b].T).astype(bf) for b in range(B)]
    wq_s = [np.ascontiguousarray(W_q[:, g * DG:(g + 1) * DG]).astype(bf)
            for g in range(2)]
    wk_s = [np.ascontiguousarray(W_k[:, g * DG:(g + 1) * DG]).astype(bf)
            for g in range(2)]
    wv_s = [np.ascontiguousarray(W_v[:, g * DG:(g + 1) * DG]).astype(bf)
            for g in range(2)]
    wo_s = [np.ascontiguousarray(W_o[g * DG:(g + 1) * DG, :]).astype(bf)
            for g in range(2)]

    in_maps = []
    for c in range(8):
        b, g = c // 2, c % 2
        in_maps.append({
            "xq": xq_t[b], "xk": xk_t[b], "xv": xv_t[b],
            "wq": wq_s[g], "wk": wk_s[g], "wv": wv_s[g], "wo": wo_s[g],
            "bq": b_q[g * DG:(g + 1) * DG],
            "bk": b_k[g * DG:(g + 1) * DG],
        })

    nc = _get_nc()
    trace = bool(int(os.environ.get("KERNEL_TRACE", "0")))
    if trace:
        try:
            import axon_profile_shim
            axon_profile_shim.install()
        except Exception:
            pass
    res = run_bass_kernel_spmd(nc, in_maps, core_ids=list(range(8)), trace=trace)
    if res.exec_time_ns is not None:
        print(f"HW exec time: {res.exec_time_ns} ns", flush=True)

    out = np.empty((B, S, D), dtype=np.float32)
    # b_v is an exact constant output offset: softmax rows sum to 1, so
    # attn @ (V + 1 b_v^T) @ W_o = attn @ V @ W_o + b_v @ W_o.
    bv_off = [b_v[g * DG:(g + 1) * DG] @ W_o[g * DG:(g + 1) * DG, :]
              for g in range(2)]
    full_bias = b_o + bv_off[0] + bv_off[1]
    for b in range(B):
        part = res.results[2 * b]["o_t"] + res.results[2 * b + 1]["o_t"]
        out[b] = part.T + full_bias
    return out



# revision 2
# speedup vs baseline: 1.0278x; 1.0278x over previous
"""Multi-head attention (B=4, S=2048, D=1024, H=16) on 8 TRN2 NeuronCores.

Sharding (data + head parallel): core c handles batch b = c//2 and head
group g = c%2 (8 of the 16 heads, feature columns 512g:512(g+1)).
Each core computes its heads' full attention locally and a partial
output projection; the host sums the two partials per batch and adds
b_o plus the b_v @ W_o term (softmax rows sum to 1, so the V bias is an
exact constant output offset and never touches the device).

v4 schedule. Steady state is ScalarE-exp-bound (256 ACTIVATEs of
[128,1024] ~1.14us each = ~292us); everything else must hide under it:
  - Head: xk fully staged (4x 1MB batched DMAs, sync ring) while wk/wq
    stream on the ACT HWDGE ring. K-proj is pb-major: pb0 for all four
    seq chunks first, interleaved with group (0,0)'s scores so the exp
    stream starts ~13us in and never stalls on the remaining K-proj.
  - PSUM: sps 2x[128,1024] (4 banks) + dedicated pv pool 2x[65,512]
    (2 banks) + mps 2x[128,512] (2 banks) = 8 banks. PV no longer
    blocks filler-matmul evacuation (was: pv shared mps).
  - pv evac: the at-divide tensor_tensor reads PV PSUM directly (the
    [65,512] staging copy is gone, -22us DVE).
  - scores^T per j-block: two K=64 row-packed matmuls (2 heads), exp
    on ScalarE from PSUM (scale=1/8 folded; no max subtraction:
    scores ~ N(0,1) so exp is safely bounded).
  - PV per head: V augmented with a ones column (M=65) so PSUM row 64
    accumulates the softmax denominator.
  - out = Wo^T @ AT per ic, bf16 partial to HBM (host sums in f32),
    spread across the next ic's groups; the tail ic uses ScalarE for
    half the evacuations and both HWDGE rings for the store.
"""

import os

import numpy as np

import concourse.bass as bass  # noqa: F401
import concourse.mybir as mybir
import concourse.tile as tile
from concourse import bacc
from concourse.bass_utils import run_bass_kernel_spmd

f32 = mybir.dt.float32
bf16 = mybir.dt.bfloat16
Exp = mybir.ActivationFunctionType.Exp
MULT = mybir.AluOpType.mult

B, S, D = 4, 2048, 1024
H_LOC = 8
DK = 64
DG = 512
KB = D // 128
PB = DG // 128
JB = S // 128
IC = S // 512
N = 512
QK_DT = bf16


def _build():
    nc = bacc.Bacc("TRN2")

    xq = nc.dram_tensor("xq", (D, S), QK_DT, kind="ExternalInput")
    xk = nc.dram_tensor("xk", (D, S), QK_DT, kind="ExternalInput")
    xv = nc.dram_tensor("xv", (D, S), bf16, kind="ExternalInput")
    wq = nc.dram_tensor("wq", (D, DG), QK_DT, kind="ExternalInput")
    wk = nc.dram_tensor("wk", (D, DG), QK_DT, kind="ExternalInput")
    wv = nc.dram_tensor("wv", (D, DG), bf16, kind="ExternalInput")
    wo = nc.dram_tensor("wo", (DG, D), bf16, kind="ExternalInput")
    bq = nc.dram_tensor("bq", (DG,), f32, kind="ExternalInput")
    bk = nc.dram_tensor("bk", (DG,), f32, kind="ExternalInput")
    o_t = nc.dram_tensor("o_t", (D, S), bf16, kind="ExternalOutput")

    with tile.TileContext(nc) as tc:
        with (
            tc.tile_pool(name="persist", bufs=1) as persist,
            tc.tile_pool(name="wp", bufs=3) as wp,
            tc.tile_pool(name="xqp", bufs=3) as xqp,
            tc.tile_pool(name="xvp", bufs=2) as xvp,
            tc.tile_pool(name="qtp", bufs=4) as qtp,
            tc.tile_pool(name="atp", bufs=6) as atp,
            tc.tile_pool(name="ptp", bufs=26) as ptp,
            tc.tile_pool(name="rbp", bufs=2) as rbp,
            tc.tile_pool(name="osb", bufs=4) as osbp,
            tc.tile_pool(name="sps", bufs=2, space="PSUM") as sps,
            tc.tile_pool(name="pvp", bufs=2, space="PSUM") as pvp,
            tc.tile_pool(name="mps", bufs=2, space="PSUM") as mps,
        ):
            # ---- persistent tensors -------------------------------------
            KT = [persist.tile([128, S], QK_DT, tag=f"kt{p}", name=f"kt{p}")
                  for p in range(PB)]
            VA = [persist.tile([128, H_LOC, DK + 1], bf16, tag=f"va{j}",
                               name=f"va{j}") for j in range(JB)]
            xk_sb = persist.tile([128, KB, S], QK_DT, tag="xks", name="xk_sb")
            for j in range(JB):
                nc.vector.memset(VA[j][:, :, DK:DK + 1], 1.0)

            bq_t = persist.tile([128, PB], f32, tag="bq")
            bk_t = persist.tile([128, PB], f32, tag="bk")
            nc.sync.dma_start(out=bq_t, in_=bq.rearrange("(pb p) -> p pb", p=128))
            nc.sync.dma_start(out=bk_t, in_=bk.rearrange("(pb p) -> p pb", p=128))

            # ---- DMA staging -------------------------------------------
            # sync ring: xk chunks (k_proj gates the exp ramp), then xq0;
            # ACT ring: wk + wq + xq1 (free until the exp stream starts).
            xq_t = {}

            def dma_xk_chunk(jc):
                nc.sync.dma_start(
                    out=xk_sb[:, :, jc * N:(jc + 1) * N],
                    in_=xk.rearrange("(kb p) n -> p kb n", p=128)[
                        :, :, jc * N:(jc + 1) * N],
                )

            def dma_xq_chunk(ic, engine):
                t = xqp.tile([128, KB, N], QK_DT, tag="xq", name="xq_c")
                engine.dma_start(
                    out=t,
                    in_=xq.rearrange("(kb p) n -> p kb n", p=128)[
                        :, :, ic * N:(ic + 1) * N],
                )
                xq_t[ic] = t

            wk_t = wp.tile([128, KB, N], QK_DT, tag="w", name="wk_t")
            nc.scalar.dma_start(
                out=wk_t, in_=wk.rearrange("(kb p) n -> p kb n", p=128)
            )
            dma_xk_chunk(0)
            dma_xq_chunk(0, nc.sync)
            wq_t = wp.tile([128, KB, N], QK_DT, tag="w", name="wq_t")
            nc.scalar.dma_start(
                out=wq_t, in_=wq.rearrange("(kb p) n -> p kb n", p=128)
            )
            for jc in range(1, IC):
                dma_xk_chunk(jc)
            dma_xq_chunk(1, nc.scalar)

            # ---- projection helpers ------------------------------------
            def k_proj(jc, pb):
                ps = mps.tile([128, N], f32, tag="mm", name="ps_k")
                for kb in range(KB):
                    nc.tensor.matmul(
                        ps,
                        wk_t[:, kb, pb * 128:(pb + 1) * 128],
                        xk_sb[:, kb, jc * N:(jc + 1) * N],
                        start=(kb == 0),
                        stop=(kb == KB - 1),
                    )
                nc.vector.tensor_scalar_add(
                    KT[pb][:, jc * N:(jc + 1) * N], ps, bk_t[:, pb:pb + 1]
                )

            def q_proj(p, ic):
                ps = mps.tile([128, N], f32, tag="mm", name="ps_q")
                for kb in range(KB):
                    nc.tensor.matmul(
                        ps,
                        wq_t[:, kb, p * 128:(p + 1) * 128],
                        xq_t[ic][:, kb, :],
                        start=(kb == 0),
                        stop=(kb == KB - 1),
                    )
                qt = qtp.tile([128, N], QK_DT, tag="qt", name="qt")
                nc.vector.tensor_scalar_add(qt, ps, bq_t[:, p:p + 1])
                return qt

            def dma_xv_chunk(jg):
                t = xvp.tile([128, KB, N], bf16, tag="xv", name="xv_c")
                nc.sync.dma_start(
                    out=t,
                    in_=xv.rearrange("(kb p) n -> p kb n", p=128)[
                        :, :, jg * N:(jg + 1) * N],
                )
                return t

            def v_proj_chunk(jg, xc=None):
                if xc is None:
                    xc = dma_xv_chunk(jg)
                for jj in range(4):
                    j = jg * 4 + jj
                    ps = mps.tile([128, N], f32, tag="mm", name="vps")
                    for kb in range(KB):
                        nc.tensor.matmul(
                            ps,
                            xc[:, kb, jj * 128:(jj + 1) * 128],
                            wv_t[:, kb, :],
                            start=(kb == 0),
                            stop=(kb == KB - 1),
                        )
                    nc.vector.tensor_copy(
                        VA[j][:, :, 0:DK],
                        ps.rearrange("p (h e) -> p h e", e=DK),
                    )

            # ---- attention phases --------------------------------------
            def scores_block(pair, j, qt):
                s_ps = sps.tile([128, 2 * N], f32, tag="s", name="s_ps")
                nc.tensor.matmul(
                    s_ps[:, 0:N],
                    KT[pair][0:64, j * 128:(j + 1) * 128],
                    qt[0:64, :],
                    start=True, stop=True,
                )
                nc.tensor.matmul(
                    s_ps[:, N:2 * N],
                    KT[pair][64:128, j * 128:(j + 1) * 128],
                    qt[64:128, :],
                    start=True, stop=True,
                    tile_position=(64, 0),
                )
                pt = ptp.tile([128, 2 * N], bf16, tag="pt", name="pt")
                nc.scalar.activation(pt, s_ps, Exp, scale=0.125)
                return pt

            def scores_group(pair, ic, qt, jset=range(JB), pts=None):
                if pts is None:
                    pts = []
                for j in jset:
                    pts.append(scores_block(pair, j, qt))
                return (pair, ic, pts)

            def pv_phase(pair, ic, pts, last=False):
                pv = [
                    pvp.tile([DK + 1, N], f32, tag="pv", name="pv0"),
                    pvp.tile([DK + 1, N], f32, tag="pv", name="pv1"),
                ]
                if last:
                    # j-outer so only the final j's matmuls trail the
                    # last exp instead of a whole serial h2=1 pass
                    for j in range(JB):
                        for h2 in range(2):
                            nc.tensor.matmul(
                                pv[h2],
                                VA[j][:, 2 * pair + h2, :],
                                pts[j][:, h2 * N:(h2 + 1) * N],
                                start=(j == 0),
                                stop=(j == JB - 1),
                                skip_group_check=True,
                            )
                else:
                    for h2 in range(2):
                        for j in range(JB):
                            nc.tensor.matmul(
                                pv[h2],
                                VA[j][:, 2 * pair + h2, :],
                                pts[j][:, h2 * N:(h2 + 1) * N],
                                start=(j == 0),
                                stop=(j == JB - 1),
                            )
                at = atp.tile([128, N], bf16, tag="at", name="at")
                for h2 in range(2):
                    den = rbp.tile([1, N], f32, tag="den", name="den")
                    nc.vector.tensor_copy(den, pv[h2][DK:DK + 1, :])
                    rbr = rbp.tile([64, N], f32, tag="rbr", name="rbr")
                    nc.gpsimd.partition_broadcast(rbr, den)
                    rb = rbp.tile([64, N], f32, tag="rb", name="rb")
                    nc.vector.reciprocal_approx_fast(rb, rbr)
                    nc.vector.tensor_tensor(
                        out=at[h2 * 64:(h2 + 1) * 64, :],
                        in0=pv[h2][0:DK, :], in1=rb, op=MULT,
                    )
                return at

            def oproj_ic(ic, at_tiles, dobs=None, tail=False):
                for dob in (range(KB) if dobs is None else dobs):
                    ops = mps.tile([128, N], f32, tag="mm", name="ops")
                    for pb in range(PB):
                        nc.tensor.matmul(
                            ops,
                            wo_t[:, pb, dob * 128:(dob + 1) * 128],
                            at_tiles[pb],
                            start=(pb == 0),
                            stop=(pb == PB - 1),
                        )
                    ob = osbp.tile([128, N], bf16, tag="ob", name="ob")
                    if tail and dob % 2 == 1:
                        # ScalarE is idle after the last exp; split the
                        # tail evacuation + store across both engines/rings
                        nc.scalar.copy(ob, ops)
                        nc.scalar.dma_start(
                            out=o_t[dob * 128:(dob + 1) * 128,
                                    ic * N:(ic + 1) * N],
                            in_=ob,
                        )
                    else:
                        nc.vector.tensor_copy(ob, ops)
                        nc.sync.dma_start(
                            out=o_t[dob * 128:(dob + 1) * 128,
                                    ic * N:(ic + 1) * N],
                            in_=ob,
                        )

            # ---- main schedule -----------------------------------------
            # K-proj pb-major: pb0 for all chunks first, group (0,0)'s
            # scores right behind each chunk so the exp stream starts
            # as soon as wk+xk-c0 (+wq+xq-c0) land, then pb1-3 fill the
            # PE under the exp stream.
            qt_next = {}
            st = None
            for jc in range(IC):
                k_proj(jc, 0)
                if jc == 0:
                    qt_next[(0, 0)] = q_proj(0, 0)
                    st = scores_group(0, 0, qt_next[(0, 0)], range(0, 4))
                else:
                    st = scores_group(0, 0, qt_next[(0, 0)],
                                      range(4 * jc, 4 * jc + 4), pts=st[2])
            for pb in range(1, PB):
                for jc in range(IC):
                    k_proj(jc, pb)
                if pb == 1:
                    qt_next[(1, 0)] = q_proj(1, 0)

            # post-head DMA: wv + the first two xv chunks
            wv_t = wp.tile([128, KB, N], bf16, tag="w", name="wv_t")
            nc.sync.dma_start(
                out=wv_t, in_=wv.rearrange("(kb p) n -> p kb n", p=128)
            )
            xv01 = [dma_xv_chunk(0), dma_xv_chunk(1)]

            groups = [(p, ic) for ic in range(IC) for p in range(PB)]
            wo_t = None
            prev = None        # unconsumed (pair, ic, pts)
            at_done = {}       # (pair, ic) -> at tile
            for gi, (pair, ic) in enumerate(groups):
                if (pair, ic) != (0, 0):
                    st = scores_group(pair, ic, qt_next.pop((pair, ic)))
                # q-proj one group ahead, right behind this group's scores
                nxt = groups[gi + 1] if gi + 1 < len(groups) else None
                if nxt is not None and nxt not in qt_next:
                    qt_next[nxt] = q_proj(*nxt)
                # ---- PE filler inside this group's exp window ----
                if ic == 0:
                    if pair == 0:
                        v_proj_chunk(0, xv01[0])
                        v_proj_chunk(1, xv01[1])
                    elif pair == 1:
                        v_proj_chunk(2)
                        v_proj_chunk(3)
                        dma_xq_chunk(2, nc.sync)
                        dma_xq_chunk(3, nc.sync)
                    elif pair == 2:
                        # wo reuses wk's pool buffer; allocate only after
                        # every k_proj read has been emitted.
                        wo_t = wp.tile([128, PB, D], bf16, tag="w",
                                       name="wo_t")
                        nc.sync.dma_start(
                            out=wo_t,
                            in_=wo.rearrange("(pb p) n -> p pb n", p=128),
                        )
                if prev is not None:
                    p_prev, ic_prev = prev[0], prev[1]
                    at_done[(p_prev, ic_prev)] = pv_phase(*prev)
                    if ic > 0:
                        # spread the output projection of ic-1 across this
                        # ic's groups so no single exp window overruns
                        ats = [at_done[(p, ic - 1)] for p in range(PB)]
                        if pair == 1:
                            oproj_ic(ic - 1, ats, range(0, 3))
                        elif pair == 2:
                            oproj_ic(ic - 1, ats, range(3, 6))
                        elif pair == 3:
                            oproj_ic(ic - 1, ats, range(6, 8))
                            for p in range(PB):
                                at_done.pop((p, ic - 1))
                prev = st
            at_done[(prev[0], prev[1])] = pv_phase(*prev, last=True)
            oproj_ic(IC - 1, [at_done.pop((p, IC - 1)) for p in range(PB)],
                     tail=True)

    nc.compile()
    return nc


_NC_CACHE = None


def _get_nc():
    global _NC_CACHE
    if _NC_CACHE is None:
        _NC_CACHE = _build()
    return _NC_CACHE


def kernel(q, k, v, W_q, b_q, W_k, b_k, W_v, b_v, W_o, b_o):
    import ml_dtypes

    q = np.asarray(q, dtype=np.float32)
    k = np.asarray(k, dtype=np.float32)
    v = np.asarray(v, dtype=np.float32)
    W_q = np.asarray(W_q, dtype=np.float32)
    W_k = np.asarray(W_k, dtype=np.float32)
    W_v = np.asarray(W_v, dtype=np.float32)
    W_o = np.asarray(W_o, dtype=np.float32)
    b_q = np.asarray(b_q, dtype=np.float32)
    b_k = np.asarray(b_k, dtype=np.float32)
    b_v = np.asarray(b_v, dtype=np.float32)
    b_o = np.asarray(b_o, dtype=np.float32)

    bf = ml_dtypes.bfloat16
    xq_t = [np.ascontiguousarray(q[b].T).astype(bf) for b in range(B)]
    xk_t = [np.ascontiguousarray(k[b].T).astype(bf) for b in range(B)]
    xv_t = [np.ascontiguousarray(v[b].T).astype(bf) for b in range(B)]
    wq_s = [np.ascontiguousarray(W_q[:, g * DG:(g + 1) * DG]).astype(bf)
            for g in range(2)]
    wk_s = [np.ascontiguousarray(W_k[:, g * DG:(g + 1) * DG]).astype(bf)
            for g in range(2)]
    wv_s = [np.ascontiguousarray(W_v[:, g * DG:(g + 1) * DG]).astype(bf)
            for g in range(2)]
    wo_s = [np.ascontiguousarray(W_o[g * DG:(g + 1) * DG, :]).astype(bf)
            for g in range(2)]

    in_maps = []
    for c in range(8):
        b, g = c // 2, c % 2
        in_maps.append({
            "xq": xq_t[b], "xk": xk_t[b], "xv": xv_t[b],
            "wq": wq_s[g], "wk": wk_s[g], "wv": wv_s[g], "wo": wo_s[g],
            "bq": b_q[g * DG:(g + 1) * DG],
            "bk": b_k[g * DG:(g + 1) * DG],
        })

    nc = _get_nc()
    trace = bool(int(os.environ.get("KERNEL_TRACE", "0")))
    if trace:
        try:
            import axon_profile_shim
            axon_profile_shim.install()
        except Exception:
            pass
    res = run_bass_kernel_spmd(nc, in_maps, core_ids=list(range(8)), trace=trace)
    if res.exec_time_ns is not None:
        print(f"HW exec time: {res.exec_time_ns} ns", flush=True)

    out = np.empty((B, S, D), dtype=np.float32)
    # b_v is an exact constant output offset: softmax rows sum to 1, so
    # attn @ (V + 1 b_v^T) @ W_o = attn @ V @ W_o + b_v @ W_o.
    bv_off = [b_v[g * DG:(g + 1) * DG] @ W_o[g * DG:(g + 1) * DG, :]
              for g in range(2)]
    full_bias = b_o + bv_off[0] + bv_off[1]
    for b in range(B):
        part = (res.results[2 * b]["o_t"].astype(np.float32)
                + res.results[2 * b + 1]["o_t"].astype(np.float32))
        out[b] = part.T + full_bias
    return out
